# revision 21
# baseline (speedup 1.0000x reference)
"""Trainium2 Bass kernel for a dense transformer block (pre-LN, causal attn, FFN).

Sharding: pure data-parallel over batch. B=128 is split into 8 slices of 16;
each NeuronCore runs the full block on its slice with replicated weights.
No collectives.

v1 design (bf16):
  - all matmul operands bf16 (weights pre-cast, activations cast on the
    PSUM->SBUF copies); rel err ~2.5e-3 vs the 2e-2 gate
  - batch elements processed in PAIRS: QKV/FFN1 stream a [128, 512]
    moving operand covering both elements, halving matmul + weight-load
    count for the weight-stationary GEMMs
  - causal attention computes only the 3 live [t1,t2] 128-blocks per
    head (N=256 + N=128 matmuls, full rate at bf16); the fully-masked
    block is never computed
  - softmax without max-subtraction (scores are O(8)); no additive mask:
    exp() runs straight out of PSUM and the two staircase diagonal
    blocks are zeroed in-place by gpsimd affine_select on P
  - softmax denominators come free from PV: the stationary operand is
    [v_h | ones-fanout] via a strided AP, so PV output rows 64:127 all
    hold sum_t2 P; one [64,T] reciprocal gives a broadcast 1/sum and a
    single TT mult normalizes while copying out of PSUM
  - PE transposes in bf16 (1 cyc/row) packed 6-per-PSUM-bank, merged to
    SBUF with one 2x-mode DVE copy per element
"""
import sys

sys.path.insert(0, "/opt/trn_rl_repo")

import numpy as np

B, T, C, H, D = 128, 256, 384, 6, 64
NCORES = 8
BL = B // NCORES  # 16 batch elements per core
EPS = 1e-5
CT = C // 128      # 3 c-tiles
FT = 4 * C // 128  # 12 f-tiles
TT = T // 128      # 2 t-tiles
VW = H * 128       # v tile width: per head [v_h (64) | ones (64)]

_cache = {}


def build(reps=1, nb=BL, loop_reps=1, skip=()):
    import concourse.bacc as bacc
    import concourse.bass as bass
    import concourse.mybir as mybir
    import concourse.tile as tile
    from concourse.masks import make_identity

    # Pin all activations to the one table that serves every function we
    # use (ln, exp, identity, copy, relu) so the kernel pays exactly one
    # ACT table load instead of thrashing 1.28us reloads.
    if not hasattr(bacc, "_orig_get_activation_tables"):
        bacc._orig_get_activation_tables = bacc.get_activation_tables

        def _pinned_tables(arch):
            t = bacc._orig_get_activation_tables(arch)
            keep = "natural_log_exp_and_others"
            assert keep in t
            return {k: (v if k == keep else set()) for k, v in t.items()}

        bacc.get_activation_tables = _pinned_tables

    F32, BF16 = mybir.dt.float32, mybir.dt.bfloat16
    AF = mybir.ActivationFunctionType
    ALU = mybir.AluOpType

    assert nb % 2 == 0
    NP = nb // 2

    nc = bacc.Bacc("TRN2", target_bir_lowering=False, debug=False)

    xd = nc.dram_tensor("x", [nb, T, C], F32, kind="ExternalInput")
    wqkv = nc.dram_tensor("wqkv", [CT, 128, 3 * C], BF16, kind="ExternalInput")
    wproj = nc.dram_tensor("wproj", [CT, 128, C], BF16, kind="ExternalInput")
    w1 = nc.dram_tensor("w1", [CT, 128, 4 * C], BF16, kind="ExternalInput")
    w2 = nc.dram_tensor("w2", [FT, 128, C], BF16, kind="ExternalInput")
    b1d = nc.dram_tensor("b1", [FT, 128], F32, kind="ExternalInput")
    qkbd = nc.dram_tensor("qkb", [2 * CT, 128], F32, kind="ExternalInput")
    vecd = nc.dram_tensor("vecs", [2, 128, C], F32, kind="ExternalInput")
    outd = nc.dram_tensor("out", [nb, T, C], F32, kind="ExternalOutput")

    with tile.TileContext(nc) as tc:
        with tc.tile_pool(name="const", bufs=1) as cp, \
             tc.tile_pool(name="acts", bufs=2) as ap, \
             tc.tile_pool(name="acts3", bufs=3) as ap3, \
             tc.tile_pool(name="small", bufs=4) as sp, \
             tc.tile_pool(name="pst", bufs=2, space="PSUM") as pst, \
             tc.tile_pool(name="ps", bufs=6, space="PSUM") as ps:

            # ---- constants ----
            wqkv_sb = cp.tile([128, CT, 3 * C], BF16)
            nc.sync.dma_start(out=wqkv_sb, in_=wqkv.rearrange("k p n -> p k n"))
            wproj_sb = cp.tile([128, CT, C], BF16)
            nc.sync.dma_start(out=wproj_sb, in_=wproj.rearrange("k p n -> p k n"))
            w1_sb = cp.tile([128, CT, 4 * C], BF16)
            nc.sync.dma_start(out=w1_sb, in_=w1.rearrange("k p n -> p k n"))
            w2_sb = cp.tile([128, FT, C], BF16)
            nc.sync.dma_start(out=w2_sb, in_=w2.rearrange("k p n -> p k n"))
            b1_sb = cp.tile([128, FT], F32)
            nc.sync.dma_start(out=b1_sb, in_=b1d.rearrange("k p -> p k"))
            qkb_sb = cp.tile([128, 2 * CT], F32)  # q/k biases per c-tile
            nc.sync.dma_start(out=qkb_sb, in_=qkbd.rearrange("k p -> p k"))
            vec_sb = cp.tile([128, 2, C], F32)  # bproj_eff, b2 bcast
            nc.sync.dma_start(out=vec_sb, in_=vecd.rearrange("k p n -> p k n"))

            ident = cp.tile([128, 128], BF16)
            make_identity(nc, ident)
            eps_sb = cp.tile([128, 1], F32)
            nc.vector.memset(eps_sb, EPS)

            BPROJ, B2 = range(2)

            def layernorm(src, dst):
                """dst(bf16) = (src - mean) * rsqrt(var + eps); token-major,
                both elements of the pair. gains/biases are folded into the
                downstream weights. rstd chains batched over the 4 rows."""
                mv4 = sp.tile([128, 2 * TT, 2], F32, tag="mv")
                rs4 = sp.tile([128, 2 * TT, 1], F32, tag="rs")
                for i in range(2 * TT):
                    st = sp.tile([128, 6], F32, tag="st")
                    nc.vector.bn_stats(out=st, in_=src[:, i, :])
                    nc.vector.bn_aggr(out=mv4[:, i, :], in_=st)
                # rstd = exp(-0.5*ln(var+eps)): keeps ACT on one table
                nc.scalar.activation(out=rs4, in_=mv4[:, :, 1:2], func=AF.Ln,
                                     bias=eps_sb, scale=1.0)
                nc.scalar.activation(out=rs4, in_=rs4, func=AF.Exp,
                                     scale=-0.5)
                for i in range(2 * TT):
                    nc.gpsimd.tensor_scalar(
                        out=dst[:, i, :], in0=src[:, i, :],
                        scalar1=mv4[:, i, 0:1], scalar2=rs4[:, i, :],
                        op0=ALU.subtract, op1=ALU.mult)

            def transpose_pair(src, dst):
                """src sbuf bf16 [128, 2*TT, C] token-major (pair) ->
                dst sbuf bf16 [128, CT, 2, T] feature-major, pair-interleaved
                so [128, ct, :, :] is a [128, 512] moving operand."""
                for e in range(2):
                    tp = pst.tile([128, 2 * CT, 128], BF16, tag="pst")
                    for ct in range(CT):
                        for tt in range(TT):
                            nc.tensor.transpose(
                                tp[:, ct * TT + tt, :],
                                src[:, e * TT + tt, ct * 128:(ct + 1) * 128],
                                ident)
                    nc.vector.tensor_copy(
                        out=dst[:, :, e, :].rearrange(
                            "p ct (tt f) -> p ct tt f", tt=TT),
                        in_=tp.rearrange("p (ct tt) f -> p ct tt f", ct=CT))

            def s_ln1(p, st):
                """x load + LN1 (DVE/ACT/gp only) - hoisted a round early.
                Also precomputes xb = x + bproj_eff (the proj-residual base)
                off the critical path."""
                x_sb = ap3.tile([128, 2 * TT, C], F32, tag="x")
                nc.sync.dma_start(
                    out=x_sb,
                    in_=xd[2 * p:2 * p + 2].rearrange(
                        "e (tt p) c -> p e tt c", p=128))
                st["x"] = x_sb
                h = ap3.tile([128, 2 * TT, C], BF16, tag="h")
                layernorm(x_sb, h)
                st["h"] = h
                # x is dead after LN1: turn it into the proj-residual base
                # xb = x + bproj_eff in place
                vb = bass.AP(vec_sb[:, :, :].tensor, BPROJ * C,
                             [[2 * C, 128], [0, 2 * TT], [1, C]])
                nc.gpsimd.tensor_add(out=x_sb, in0=x_sb, in1=vb)
                st["xb"] = x_sb

            def s_tr1(p, st):
                h_T = ap.tile([128, CT, 2, T], BF16, tag="hT")
                transpose_pair(st["h"], h_T)
                st["hT"] = h_T

            def s_qkv(p, st):
                """QKV for the pair: q,k feature-major (weights stationary,
                N=512 moving over both elements), v token-major + ones cols."""
                h_T = st["hT"]
                qk = ap.tile([128, 2 * CT, 2, T], BF16, tag="qk")
                for jt in range(2 * CT):  # q_T c-tiles 0-2, k_T 3-5
                    acc = ps.tile([128, 512], F32, tag="ps")
                    for kt in range(CT):
                        nc.tensor.matmul(
                            acc,
                            wqkv_sb[:, kt, jt * 128:(jt + 1) * 128],
                            h_T[:, kt, :, :],
                            start=(kt == 0), stop=(kt == CT - 1))
                    nc.scalar.activation(out=qk[:, jt, :, :], in_=acc,
                                         func=AF.Identity,
                                         bias=qkb_sb[:, jt:jt + 1])
                vs = []
                for e in range(2):
                    v = ap.tile([128, TT, VW], BF16, tag=f"v{e}")
                    vfull = v[:, :, :]
                    # ones in cols h*128+64 .. h*128+128 of every head block
                    ones_dst = bass.AP(vfull.tensor, 64,
                                       [[TT * VW, 128], [VW, TT],
                                        [128, H], [1, 64]])
                    nc.gpsimd.memset(ones_dst, 1.0)
                    for tt in range(TT):
                        acc = ps.tile([128, C], F32, tag="ps")
                        for kt in range(CT):
                            nc.tensor.matmul(
                                acc,
                                h_T[:, kt, e, tt * 128:(tt + 1) * 128],
                                wqkv_sb[:, kt, 2 * C:3 * C],
                                start=(kt == 0), stop=(kt == CT - 1))
                        vdst = bass.AP(vfull.tensor, tt * VW,
                                       [[TT * VW, 128], [128, H], [1, D]])
                        nc.scalar.copy(
                            out=vdst,
                            in_=acc[:, :].rearrange("p (h w) -> p h w", h=H))
                    vs.append(v)
                st["qk"], st["v"] = qk, vs

            def s_attn(p, st, e):
                """Scores + exp + causal staircase for element e of pair p.
                pt layout per head: [128(t2), 384] = (tt2=0: t1 0:256,
                tt2=1: t1 128:256); the dead (tt2=1, t1 0:128) block is
                never computed."""
                qk = st["qk"]
                pt = ap.tile([128, H, 384], BF16, tag=f"pt{e}")
                st[f"pt{e}"] = pt
                for pr in range(CT):
                    # interleave the pair's matmuls: the two heads sit in
                    # disjoint PE row groups (0:64 / 64:128) and overlap
                    sps = []
                    for u in range(2):
                        s_ps = ps.tile([128, 384], F32, tag="ps",
                                       name=f"s_ps{u}")
                        sps.append(s_ps)
                    for tt2 in range(TT):
                        for u in range(2):
                            po = 64 * u
                            if tt2 == 0:
                                nc.tensor.matmul(
                                    sps[u][:, 0:T],
                                    qk[po:po + 64, CT + pr, e, 0:128],
                                    qk[po:po + 64, pr, e, :],
                                    start=True, stop=True)
                            else:
                                nc.tensor.matmul(
                                    sps[u][:, T:T + 128],
                                    qk[po:po + 64, CT + pr, e, 128:256],
                                    qk[po:po + 64, pr, e, 128:256],
                                    start=True, stop=True)
                    for u in range(2):
                        h_i = 2 * pr + u
                        nc.scalar.activation(out=pt[:, h_i, :], in_=sps[u],
                                             func=AF.Exp, scale=D ** -0.5)
                        # zero the upper staircase of both diagonal blocks
                        # (cols 0:128 and 256:384) in one op
                        sel = bass.AP(pt[:, :, :].tensor, h_i * 384,
                                      [[H * 384, 128], [T, 2], [1, 128]])
                        nc.gpsimd.affine_select(
                            out=sel, in_=sel,
                            compare_op=ALU.is_ge, fill=0.0,
                            base=0, pattern=[[0, 2], [1, 128]],
                            channel_multiplier=-1)

            def s_pv(p, st, e):
                """PV with ones-fanout stationary: a_ps rows 0:63 = attout_h
                (unnormalized), rows 64:127 = broadcast softmax denominator.
                One reciprocal + one TT mult per head normalizes."""
                pt, v = st[f"pt{e}"], st["v"][e]
                attout = ap.tile([128, CT, T], BF16, tag=f"ao{e}")
                recb = ap.tile([64, H, T], BF16, tag=f"rec{e}")
                st[f"ao{e}"] = attout
                for pr in range(CT):  # head pair (2*pr, 2*pr+1), one bank
                    a_ps = ps.tile([128, 2, T], F32, tag="ps")
                    for u in range(2):
                        h_i = 2 * pr + u
                        for tt2 in range(TT):
                            stat = v[:, tt2, h_i * 128:(h_i + 1) * 128]
                            if tt2 == 0:
                                nc.tensor.matmul(
                                    a_ps[:, u, :], stat, pt[:, h_i, 0:T],
                                    start=True, stop=True)
                            else:
                                nc.tensor.matmul(
                                    a_ps[:, u, 128:T], stat,
                                    pt[:, h_i, T:T + 128],
                                    start=False, stop=True,
                                    skip_group_check=True)
                    if pr != 1:
                        with nc.allow_low_precision(
                                reason="softmax 1/sum feeds bf16 normalize"):
                            nc.vector.reciprocal(
                                out=recb[:, 2 * pr:2 * pr + 2, :],
                                in_=a_ps[64:128, :, :])
                    else:
                        # 1/x = exp(-ln(x)) on ACT (sums are positive);
                        # balances DVE vs ACT load
                        lns = sp.tile([64, 2, T], F32, tag="lns")
                        nc.scalar.activation(out=lns, in_=a_ps[64:128, :, :],
                                             func=AF.Ln)
                        with nc.allow_low_precision(
                                reason="softmax 1/sum feeds bf16 normalize"):
                            nc.scalar.activation(
                                out=recb[:, 2 * pr:2 * pr + 2, :],
                                in_=lns, func=AF.Exp, scale=-1.0)
                    for u in range(2):
                        h_i = 2 * pr + u
                        nc.vector.tensor_mul(
                            out=attout[64 * u:64 * u + 64, pr, :],
                            in0=a_ps[0:64, u, :], in1=recb[:, h_i, :])

            def s_proj(p, st, e):
                """proj + residual -> x1 (f32, token-major). The bproj_eff
                bias is pre-added into xb by s_ln1."""
                attout, xb = st[f"ao{e}"], st["xb"]
                if "x1" not in st:
                    x1 = ap.tile([128, 2 * TT, C], F32, tag="x1")
                    st["x1"] = x1
                x1 = st["x1"]
                for tt in range(TT):
                    p_ps = ps.tile([128, C], F32, tag="ps")
                    for ct in range(CT):
                        nc.tensor.matmul(
                            p_ps,
                            attout[:, ct, tt * 128:(tt + 1) * 128],
                            wproj_sb[:, ct, :],
                            start=(ct == 0), stop=(ct == CT - 1))
                    i = e * TT + tt
                    nc.vector.tensor_add(out=x1[:, i, :],
                                         in0=xb[:, i, :], in1=p_ps)

            def s_ln2(p, st):
                h2 = ap3.tile([128, 2 * TT, C], BF16, tag="h")
                layernorm(st["x1"], h2)
                st["h2"] = h2
                # x1 is dead after LN2: pre-add b2 in place to form the
                # ffn2 residual base, off the critical path
                vb = bass.AP(vec_sb[:, :, :].tensor, B2 * C,
                             [[2 * C, 128], [0, 2 * TT], [1, C]])
                nc.gpsimd.tensor_add(out=st["x1"], in0=st["x1"], in1=vb)
                st["xb2"] = st["x1"]

            def s_tr2(p, st):
                h2_T = ap.tile([128, CT, 2, T], BF16, tag="h2T")
                transpose_pair(st["h2"], h2_T)
                st["h2T"] = h2_T

            def s_ffn1(p, st):
                """FFN1 for the pair (N=512 moving), relu+bias on the
                PSUM->SBUF copy, alternating ACT/DVE."""
                h2_T = st["h2T"]
                ff = ap.tile([128, FT, 2, T], BF16, tag="ff")
                st["ff"] = ff
                for ft in range(FT):
                    acc = ps.tile([128, 512], F32, tag="ps")
                    for kt in range(CT):
                        nc.tensor.matmul(
                            acc,
                            w1_sb[:, kt, ft * 128:(ft + 1) * 128],
                            h2_T[:, kt, :, :],
                            start=(kt == 0), stop=(kt == CT - 1))
                    if ft % 2 == 0:
                        nc.vector.tensor_scalar(
                            out=ff[:, ft, :, :], in0=acc,
                            scalar1=b1_sb[:, ft:ft + 1], scalar2=0.0,
                            op0=ALU.add, op1=ALU.max)
                    else:
                        nc.scalar.activation(out=ff[:, ft, :, :], in_=acc,
                                             func=AF.Relu,
                                             bias=b1_sb[:, ft:ft + 1],
                                             scale=1.0)

            def s_ffn2(p, st, e):
                """FFN2 + residual + store for element e. b2 is pre-added
                into xb2 by s_ln2; the pair's output DMA goes out once."""
                ff, xb2 = st["ff"], st["xb2"]
                if "o" not in st:
                    o_sb = ap.tile([128, 2 * TT, C], F32, tag="o")
                    st["o"] = o_sb
                o_sb = st["o"]
                for tt in range(TT):
                    f_ps = ps.tile([128, C], F32, tag="ps")
                    for ft in range(FT):
                        nc.tensor.matmul(
                            f_ps,
                            ff[:, ft, e, tt * 128:(tt + 1) * 128],
                            w2_sb[:, ft, :],
                            start=(ft == 0), stop=(ft == FT - 1))
                    i = e * TT + tt
                    nc.vector.tensor_add(out=o_sb[:, i, :],
                                         in0=xb2[:, i, :], in1=f_ps)
                if e == 1:
                    nc.sync.dma_start(
                        out=outd[2 * p:2 * p + 2].rearrange(
                            "e (tt p) c -> p e tt c", p=128),
                        in_=o_sb)

            def emit_all():
                # two-pair software pipeline: pair p attention interleaves
                # with pair p-1 FFN so the PE always has cross-stream work
                states = {0: {}}
                s_ln1(0, states[0])
                for p in range(NP):
                    st = states[p]
                    prev = states.get(p - 1)
                    if prev is not None:
                        s_ln2(p - 1, prev)
                    s_tr1(p, st)
                    s_qkv(p, st)
                    if p + 1 < NP:
                        states[p + 1] = {}
                        s_ln1(p + 1, states[p + 1])
                    if prev is not None:
                        s_tr2(p - 1, prev)
                    s_attn(p, st, 0)
                    if prev is not None:
                        s_ffn1(p - 1, prev)
                    s_attn(p, st, 1)
                    s_pv(p, st, 0)
                    if prev is not None:
                        s_ffn2(p - 1, prev, 0)
                    s_pv(p, st, 1)
                    if prev is not None:
                        s_ffn2(p - 1, prev, 1)
                        del states[p - 1]
                    s_proj(p, st, 0)
                    s_proj(p, st, 1)
                last = states[NP - 1]
                s_ln2(NP - 1, last)
                s_tr2(NP - 1, last)
                s_ffn1(NP - 1, last)
                s_ffn2(NP - 1, last, 0)
                s_ffn2(NP - 1, last, 1)

            if loop_reps > 1:
                with tc.For_i(0, loop_reps, 1):
                    for _ in range(reps):
                        emit_all()
            else:
                for _ in range(reps):
                    emit_all()

    nc.compile()
    return nc


def _prep_maps(x, Wqkv, Wproj, bproj, W1, b1, W2, b2, g1, be1, g2, be2,
               nb=BL):
    import ml_dtypes
    f32 = np.float32
    f64 = np.float64
    bf16 = ml_dtypes.bfloat16
    Wqkv, Wproj = np.asarray(Wqkv, f64), np.asarray(Wproj, f64)
    W1, W2 = np.asarray(W1, f64), np.asarray(W2, f64)
    g1, be1 = np.asarray(g1, f64), np.asarray(be1, f64)
    g2, be2 = np.asarray(g2, f64), np.asarray(be2, f64)
    bproj, b1, b2 = (np.asarray(bproj, f64), np.asarray(b1, f64),
                     np.asarray(b2, f64))
    # fold LN gains into the consuming weights, LN betas into biases:
    #   h = z*g + be  =>  h @ W.T = z @ (W*g).T + (W @ be)
    Wqkv_g = Wqkv * g1[None, :]
    b_qkv = Wqkv @ be1                       # [3C]; q,k parts applied at copy
    bproj_eff = bproj + Wproj @ b_qkv[2 * C:]  # v bias folded via softmax sum=1
    W1_g = W1 * g2[None, :]
    b1_eff = b1 + W1 @ be2
    bcast = lambda v: np.ascontiguousarray(
        np.broadcast_to(np.asarray(v, f32), (128, C)))
    vecs = np.stack([bcast(bproj_eff), bcast(b2)])  # [2,128,C]
    shared = {
        "wqkv": np.ascontiguousarray(Wqkv_g.astype(bf16).T).reshape(
            CT, 128, 3 * C),
        "wproj": np.ascontiguousarray(Wproj.astype(bf16).T).reshape(
            CT, 128, C),
        "w1": np.ascontiguousarray(W1_g.astype(bf16).T).reshape(
            CT, 128, 4 * C),
        "w2": np.ascontiguousarray(W2.astype(bf16).T).reshape(FT, 128, C),
        "b1": np.ascontiguousarray(b1_eff.astype(f32).reshape(FT, 128)),
        "qkb": np.ascontiguousarray(b_qkv[:2 * C].astype(f32).reshape(
            2 * CT, 128)),
        "vecs": vecs,
    }
    x = np.asarray(x, f32)
    return [dict(shared, x=np.ascontiguousarray(x[i * nb:(i + 1) * nb]))
            for i in range(NCORES)]


def run(inputs, reps=1, trace=False, nb=BL):
    from concourse import bass_utils
    key = ("nc", reps, nb)
    if key not in _cache:
        _cache[key] = build(reps, nb)
    nc = _cache[key]
    in_maps = _prep_maps(**inputs, nb=nb)
    res = bass_utils.run_bass_kernel_spmd(
        nc, in_maps, core_ids=list(range(NCORES)), trace=trace)
    out = np.concatenate([res.results[i]["out"] for i in range(NCORES)], axis=0)
    return out, res


def kernel(**inputs):
    out, _ = run(inputs)
    return out


# ---------- cached jitted runner for benchmarking (execute-only calls) ----------
def get_runner(reps=1, nb=BL, loop_reps=1, skip=()):
    """Returns (call, put) where put(in_maps) -> device args and call(args)
    executes the prebuilt NEFF on 8 cores, returning jax output arrays.
    Mirrors bass2jax.run_bass_via_pjrt but with a persistent jit cache."""
    import jax
    import numpy as _np
    from jax.experimental.shard_map import shard_map
    from jax.sharding import Mesh, PartitionSpec, NamedSharding
    from concourse import bass2jax as B2J
    import concourse.mybir as mybir

    key = ("runner", reps, nb, loop_reps, tuple(skip))
    if key in _cache:
        return _cache[key]
    nckey = ("nc", reps, nb, loop_reps, tuple(skip))
    if nckey not in _cache:
        _cache[nckey] = build(reps, nb, loop_reps=loop_reps, skip=skip)
    nc = _cache[nckey]

    B2J.install_neuronx_cc_hook()
    part_name = (nc.partition_id_tensor.name if nc.partition_id_tensor
                 else None)
    in_names, out_names, out_avals, zero_outs = [], [], [], []
    for alloc in nc.m.functions[0].allocations:
        if not isinstance(alloc, mybir.MemoryLocationSet):
            continue
        name = alloc.memorylocations[0].name
        if alloc.kind == "ExternalInput":
            if name != part_name:
                in_names.append(name)
        elif alloc.kind == "ExternalOutput":
            out_names.append(name)
            shape = tuple(alloc.tensor_shape)
            dtype = mybir.dt.np(alloc.dtype)
            out_avals.append(jax.core.ShapedArray(shape, dtype))
            zero_outs.append(_np.zeros(shape, dtype))
    n_params = len(in_names)
    all_names = in_names + out_names
    if part_name is not None:
        all_names = all_names + [part_name]

    def _body(*args):
        operands = list(args)
        if part_name is not None:
            operands.append(B2J.partition_id_tensor())
        outs = B2J._bass_exec_p.bind(
            *operands,
            out_avals=tuple(out_avals),
            in_names=tuple(all_names),
            out_names=tuple(out_names),
            lowering_input_output_aliases=(),
            sim_require_finite=True,
            sim_require_nnan=True,
            nc=nc,
        )
        return tuple(outs)

    devices = jax.devices()[:NCORES]
    mesh = Mesh(_np.asarray(devices), ("core",))
    spec = PartitionSpec("core")
    n_outs = len(out_names)
    sharded = jax.jit(
        shard_map(_body, mesh=mesh, in_specs=(spec,) * (n_params + n_outs),
                  out_specs=(spec,) * n_outs, check_rep=False),
        keep_unused=True)
    sharding = NamedSharding(mesh, spec)

    def put(in_maps):
        args = []
        for i, name in enumerate(in_names):
            cat = _np.concatenate([_np.asarray(m[name]) for m in in_maps], 0)
            args.append(jax.device_put(cat, sharding))
        for z in zero_outs:
            cat = _np.zeros((NCORES * z.shape[0], *z.shape[1:]), z.dtype)
            args.append(jax.device_put(cat, sharding))
        return args

    def call(args):
        outs = sharded(*args)
        jax.block_until_ready(outs)
        return outs

    _cache[key] = (call, put)
    return call, put


# revision 30
# speedup vs baseline: 1.7701x; 1.7701x over previous
"""Trainium2 Bass kernel for a dense transformer block (pre-LN, causal attn, FFN).

Sharding: pure data-parallel over batch. B=128 is split into 8 slices of 16;
each NeuronCore runs the full block on its slice with replicated weights.
No collectives.

v1 design (bf16):
  - all matmul operands bf16 (weights pre-cast, activations cast on the
    PSUM->SBUF copies); rel err ~2.5e-3 vs the 2e-2 gate
  - batch elements processed in PAIRS: QKV/FFN1 stream a [128, 512]
    moving operand covering both elements, halving matmul + weight-load
    count for the weight-stationary GEMMs
  - causal attention computes only the 3 live [t1,t2] 128-blocks per
    head (N=256 + N=128 matmuls, full rate at bf16); the fully-masked
    block is never computed
  - softmax without max-subtraction (scores are O(8)); no additive mask:
    exp() runs straight out of PSUM and the two staircase diagonal
    blocks are zeroed in-place by gpsimd affine_select on P
  - softmax denominators come free from PV: the stationary operand is
    [v_h | ones-fanout] via a strided AP, so PV output rows 64:127 all
    hold sum_t2 P; one [64,T] reciprocal gives a broadcast 1/sum and a
    single TT mult normalizes while copying out of PSUM
  - PE transposes in bf16 (1 cyc/row) packed 6-per-PSUM-bank, merged to
    SBUF with one 2x-mode DVE copy per element
"""
import sys

sys.path.insert(0, "/opt/trn_rl_repo")

import numpy as np

B, T, C, H, D = 128, 256, 384, 6, 64
NCORES = 8
BL = B // NCORES  # 16 batch elements per core
EPS = 1e-5
CT = C // 128      # 3 c-tiles
FT = 4 * C // 128  # 12 f-tiles
TT = T // 128      # 2 t-tiles
VW = H * 128       # v tile width: per head [v_h (64) | ones (64)]

_cache = {}


def build(reps=1, nb=BL, loop_reps=1, skip=()):
    import concourse.bacc as bacc
    import concourse.bass as bass
    import concourse.mybir as mybir
    import concourse.tile as tile
    from concourse.masks import make_identity

    # Pin all activations to the one table that serves every function we
    # use (ln, exp, identity, copy, relu) so the kernel pays exactly one
    # ACT table load instead of thrashing 1.28us reloads.
    if not hasattr(bacc, "_orig_get_activation_tables"):
        bacc._orig_get_activation_tables = bacc.get_activation_tables

        def _pinned_tables(arch):
            t = bacc._orig_get_activation_tables(arch)
            keep = "natural_log_exp_and_others"
            assert keep in t
            return {k: (v if k == keep else set()) for k, v in t.items()}

        bacc.get_activation_tables = _pinned_tables

    F32, BF16 = mybir.dt.float32, mybir.dt.bfloat16
    AF = mybir.ActivationFunctionType
    ALU = mybir.AluOpType

    assert nb % 2 == 0
    NP = nb // 2

    nc = bacc.Bacc("TRN2", target_bir_lowering=False, debug=False)

    xd = nc.dram_tensor("x", [nb, T, C], F32, kind="ExternalInput")
    wqkv = nc.dram_tensor("wqkv", [CT, 128, 3 * C], BF16, kind="ExternalInput")
    wproj = nc.dram_tensor("wproj", [CT, 128, C], BF16, kind="ExternalInput")
    w1 = nc.dram_tensor("w1", [CT, 128, 4 * C], BF16, kind="ExternalInput")
    w2 = nc.dram_tensor("w2", [FT, 128, C], BF16, kind="ExternalInput")
    b1d = nc.dram_tensor("b1", [FT, 128], F32, kind="ExternalInput")
    qkbd = nc.dram_tensor("qkb", [2 * CT, 128], F32, kind="ExternalInput")
    browd = nc.dram_tensor("brows", [1, 2, C], BF16, kind="ExternalInput")
    outd = nc.dram_tensor("out", [nb, T, C], F32, kind="ExternalOutput")

    with tile.TileContext(nc) as tc:
        with tc.tile_pool(name="const", bufs=1) as cp, \
             tc.tile_pool(name="acts", bufs=2) as ap, \
             tc.tile_pool(name="acts3", bufs=3) as ap3, \
             tc.tile_pool(name="small", bufs=4) as sp, \
             tc.tile_pool(name="pst", bufs=2, space="PSUM") as pst, \
             tc.tile_pool(name="ps", bufs=6, space="PSUM") as ps:

            # ---- constants ----
            wqkv_sb = cp.tile([128, CT, 3 * C], BF16)
            nc.sync.dma_start(out=wqkv_sb, in_=wqkv.rearrange("k p n -> p k n"))
            wproj_sb = cp.tile([128, CT, C], BF16)
            nc.sync.dma_start(out=wproj_sb, in_=wproj.rearrange("k p n -> p k n"))
            w1_sb = cp.tile([128, CT, 4 * C], BF16)
            nc.sync.dma_start(out=w1_sb, in_=w1.rearrange("k p n -> p k n"))
            w2_sb = cp.tile([128, FT, C], BF16)
            nc.sync.dma_start(out=w2_sb, in_=w2.rearrange("k p n -> p k n"))
            b1_sb = cp.tile([128, FT], F32)
            nc.sync.dma_start(out=b1_sb, in_=b1d.rearrange("k p -> p k"))
            qkb_sb = cp.tile([128, 2 * CT], F32)  # q/k biases per c-tile
            nc.sync.dma_start(out=qkb_sb, in_=qkbd.rearrange("k p -> p k"))
            brow_sb = cp.tile([1, 2, C], BF16)  # bproj_eff, b2 rows
            nc.sync.dma_start(out=brow_sb, in_=browd[:, :, :])
            onesrow = cp.tile([1, 128], BF16)
            nc.vector.memset(onesrow, 1.0)

            ident = cp.tile([128, 128], BF16)
            make_identity(nc, ident)
            eps_sb = cp.tile([128, 1], F32)
            nc.vector.memset(eps_sb, EPS)

            BPROJ, B2 = range(2)

            def layernorm(src, dst):
                """dst(bf16) = (src - mean) * rsqrt(var + eps); token-major,
                both elements of the pair. gains/biases are folded into the
                downstream weights. rstd chains batched over the 4 rows."""
                mv4 = sp.tile([128, 2 * TT, 2], F32, tag="mv")
                rs4 = sp.tile([128, 2 * TT, 1], F32, tag="rs")
                for i in range(2 * TT):
                    st = sp.tile([128, 6], F32, tag="st")
                    nc.vector.bn_stats(out=st, in_=src[:, i, :])
                    nc.vector.bn_aggr(out=mv4[:, i, :], in_=st)
                # rstd = exp(-0.5*ln(var+eps)): keeps ACT on one table
                nc.scalar.activation(out=rs4, in_=mv4[:, :, 1:2], func=AF.Ln,
                                     bias=eps_sb, scale=1.0)
                nc.scalar.activation(out=rs4, in_=rs4, func=AF.Exp,
                                     scale=-0.5)
                for i in range(2 * TT):
                    nc.vector.tensor_scalar(
                        out=dst[:, i, :], in0=src[:, i, :],
                        scalar1=mv4[:, i, 0:1], scalar2=rs4[:, i, :],
                        op0=ALU.subtract, op1=ALU.mult)

            def transpose_pair(src, dst):
                """src sbuf bf16 [128, 2*TT, C] token-major (pair) ->
                dst sbuf bf16 [128, CT, 2, T] feature-major, pair-interleaved
                so [128, ct, :, :] is a [128, 512] moving operand."""
                for e in range(2):
                    tp = pst.tile([128, 2 * CT, 128], BF16, tag="pst")
                    for ct in range(CT):
                        for tt in range(TT):
                            nc.tensor.transpose(
                                tp[:, ct * TT + tt, :],
                                src[:, e * TT + tt, ct * 128:(ct + 1) * 128],
                                ident)
                    nc.vector.tensor_copy(
                        out=dst[:, :, e, :].rearrange(
                            "p ct (tt f) -> p ct tt f", tt=TT),
                        in_=tp.rearrange("p (ct tt) f -> p ct tt f", ct=CT))

            def s_ln1(p, st):
                """x load + LN1 (DVE/ACT/gp only) - hoisted a round early.
                Also precomputes xb = x + bproj_eff (the proj-residual base)
                off the critical path."""
                x_sb = ap3.tile([128, 2 * TT, C], F32, tag="x")
                nc.sync.dma_start(
                    out=x_sb,
                    in_=xd[2 * p:2 * p + 2].rearrange(
                        "e (tt p) c -> p e tt c", p=128))
                st["x"] = x_sb
                h = ap3.tile([128, 2 * TT, C], BF16, tag="h")
                layernorm(x_sb, h)
                st["h"] = h

            def s_tr1(p, st):
                h_T = ap.tile([128, CT, 2, T], BF16, tag="hT")
                transpose_pair(st["h"], h_T)
                st["hT"] = h_T

            def s_qkv(p, st):
                """QKV for the pair: q,k feature-major (weights stationary,
                N=512 moving over both elements), v token-major + ones cols."""
                h_T = st["hT"]
                qk = ap.tile([128, 2 * CT, 2, T], BF16, tag="qk")
                for jt in range(2 * CT):  # q_T c-tiles 0-2, k_T 3-5
                    acc = ps.tile([128, 512], F32, tag="ps")
                    for kt in range(CT):
                        nc.tensor.matmul(
                            acc,
                            wqkv_sb[:, kt, jt * 128:(jt + 1) * 128],
                            h_T[:, kt, :, :],
                            start=(kt == 0), stop=(kt == CT - 1))
                    nc.scalar.activation(out=qk[:, jt, :, :], in_=acc,
                                         func=AF.Identity,
                                         bias=qkb_sb[:, jt:jt + 1])
                vs = []
                for e in range(2):
                    v = ap.tile([128, TT, VW], BF16, tag=f"v{e}")
                    vfull = v[:, :, :]
                    # ones in cols h*128+64 .. h*128+128 of every head block
                    ones_dst = bass.AP(vfull.tensor, 64,
                                       [[TT * VW, 128], [VW, TT],
                                        [128, H], [1, 64]])
                    nc.gpsimd.memset(ones_dst, 1.0)
                    for tt in range(TT):
                        acc = ps.tile([128, C], F32, tag="ps")
                        for kt in range(CT):
                            nc.tensor.matmul(
                                acc,
                                h_T[:, kt, e, tt * 128:(tt + 1) * 128],
                                wqkv_sb[:, kt, 2 * C:3 * C],
                                start=(kt == 0), stop=(kt == CT - 1))
                        vdst = bass.AP(vfull.tensor, tt * VW,
                                       [[TT * VW, 128], [128, H], [1, D]])
                        nc.scalar.copy(
                            out=vdst,
                            in_=acc[:, :].rearrange("p (h w) -> p h w", h=H))
                    vs.append(v)
                st["qk"], st["v"] = qk, vs

            def s_attn(p, st, e):
                """Scores + exp + causal staircase for element e of pair p.
                pt layout per head: [128(t2), 384] = (tt2=0: t1 0:256,
                tt2=1: t1 128:256); the dead (tt2=1, t1 0:128) block is
                never computed."""
                qk = st["qk"]
                pt = ap.tile([128, H, 384], BF16, tag=f"pt{e}")
                st[f"pt{e}"] = pt
                for pr in range(CT):
                    # interleave the pair's matmuls: the two heads sit in
                    # disjoint PE row groups (0:64 / 64:128) and overlap
                    sps = []
                    for u in range(2):
                        s_ps = ps.tile([128, 384], F32, tag="ps",
                                       name=f"s_ps{u}")
                        sps.append(s_ps)
                    for tt2 in range(TT):
                        for u in range(2):
                            po = 64 * u
                            if tt2 == 0:
                                nc.tensor.matmul(
                                    sps[u][:, 0:T],
                                    qk[po:po + 64, CT + pr, e, 0:128],
                                    qk[po:po + 64, pr, e, :],
                                    start=True, stop=True)
                            else:
                                nc.tensor.matmul(
                                    sps[u][:, T:T + 128],
                                    qk[po:po + 64, CT + pr, e, 128:256],
                                    qk[po:po + 64, pr, e, 128:256],
                                    start=True, stop=True)
                    for u in range(2):
                        h_i = 2 * pr + u
                        nc.scalar.activation(out=pt[:, h_i, :], in_=sps[u],
                                             func=AF.Exp, scale=D ** -0.5)
                        # zero the upper staircase of both diagonal blocks
                        # (cols 0:128 and 256:384) in one op
                        sel = bass.AP(pt[:, :, :].tensor, h_i * 384,
                                      [[H * 384, 128], [T, 2], [1, 128]])
                        nc.gpsimd.affine_select(
                            out=sel, in_=sel,
                            compare_op=ALU.is_ge, fill=0.0,
                            base=0, pattern=[[0, 2], [1, 128]],
                            channel_multiplier=-1)

            def s_pv(p, st, e):
                """PV with ones-fanout stationary: a_ps rows 0:63 = attout_h
                (unnormalized), rows 64:127 = broadcast softmax denominator.
                One reciprocal + one TT mult per head normalizes."""
                pt, v = st[f"pt{e}"], st["v"][e]
                attout = ap.tile([128, CT, T], BF16, tag=f"ao{e}")
                recb = ap.tile([64, H, T], BF16, tag=f"rec{e}")
                st[f"ao{e}"] = attout
                for pr in range(CT):  # head pair (2*pr, 2*pr+1), one bank
                    a_ps = ps.tile([128, 2, T], F32, tag="ps")
                    for u in range(2):
                        h_i = 2 * pr + u
                        for tt2 in range(TT):
                            stat = v[:, tt2, h_i * 128:(h_i + 1) * 128]
                            if tt2 == 0:
                                nc.tensor.matmul(
                                    a_ps[:, u, :], stat, pt[:, h_i, 0:T],
                                    start=True, stop=True)
                            else:
                                nc.tensor.matmul(
                                    a_ps[:, u, 128:T], stat,
                                    pt[:, h_i, T:T + 128],
                                    start=False, stop=True,
                                    skip_group_check=True)
                    if pr != 1:
                        with nc.allow_low_precision(
                                reason="softmax 1/sum feeds bf16 normalize"):
                            nc.vector.reciprocal(
                                out=recb[:, 2 * pr:2 * pr + 2, :],
                                in_=a_ps[64:128, :, :])
                    else:
                        # 1/x = exp(-ln(x)) on ACT (sums are positive);
                        # balances DVE vs ACT load
                        lns = sp.tile([64, 2, T], F32, tag="lns")
                        nc.scalar.activation(out=lns, in_=a_ps[64:128, :, :],
                                             func=AF.Ln)
                        with nc.allow_low_precision(
                                reason="softmax 1/sum feeds bf16 normalize"):
                            nc.scalar.activation(
                                out=recb[:, 2 * pr:2 * pr + 2, :],
                                in_=lns, func=AF.Exp, scale=-1.0)
                    for u in range(2):
                        h_i = 2 * pr + u
                        nc.vector.tensor_mul(
                            out=attout[64 * u:64 * u + 64, pr, :],
                            in0=a_ps[0:64, u, :], in1=recb[:, h_i, :])

            def s_proj(p, st, e):
                """proj + residual -> x1 (f32, token-major). bproj_eff rides
                the accumulation as a K=1 ones-row matmul."""
                attout, x_sb = st[f"ao{e}"], st["x"]
                if "x1" not in st:
                    x1 = ap.tile([128, 2 * TT, C], F32, tag="x1")
                    st["x1"] = x1
                x1 = st["x1"]
                for tt in range(TT):
                    p_ps = ps.tile([128, C], F32, tag="ps")
                    for ct in range(CT):
                        nc.tensor.matmul(
                            p_ps,
                            attout[:, ct, tt * 128:(tt + 1) * 128],
                            wproj_sb[:, ct, :],
                            start=(ct == 0), stop=False)
                    nc.tensor.matmul(
                        p_ps, onesrow, brow_sb[:, BPROJ, :],
                        start=False, stop=True)
                    i = e * TT + tt
                    nc.vector.tensor_add(out=x1[:, i, :],
                                         in0=x_sb[:, i, :], in1=p_ps)

            def s_ln2(p, st):
                h2 = ap3.tile([128, 2 * TT, C], BF16, tag="h")
                layernorm(st["x1"], h2)
                st["h2"] = h2

            def s_tr2(p, st):
                h2_T = ap.tile([128, CT, 2, T], BF16, tag="h2T")
                transpose_pair(st["h2"], h2_T)
                st["h2T"] = h2_T

            def s_ffn1(p, st):
                """FFN1 for the pair (N=512 moving), relu+bias on the
                PSUM->SBUF copy, alternating ACT/DVE."""
                h2_T = st["h2T"]
                ff = ap.tile([128, FT, 2, T], BF16, tag="ff")
                st["ff"] = ff
                for ft in range(FT):
                    acc = ps.tile([128, 512], F32, tag="ps")
                    for kt in range(CT):
                        nc.tensor.matmul(
                            acc,
                            w1_sb[:, kt, ft * 128:(ft + 1) * 128],
                            h2_T[:, kt, :, :],
                            start=(kt == 0), stop=(kt == CT - 1))
                    if ft % 2 == 0:
                        nc.vector.tensor_scalar(
                            out=ff[:, ft, :, :], in0=acc,
                            scalar1=b1_sb[:, ft:ft + 1], scalar2=0.0,
                            op0=ALU.add, op1=ALU.max)
                    else:
                        nc.scalar.activation(out=ff[:, ft, :, :], in_=acc,
                                             func=AF.Relu,
                                             bias=b1_sb[:, ft:ft + 1],
                                             scale=1.0)

            def s_ffn2(p, st, e):
                """FFN2 + residual + store for element e. b2 rides the
                accumulation as a K=1 ones-row matmul."""
                ff, x1 = st["ff"], st["x1"]
                if "o" not in st:
                    o_sb = ap.tile([128, 2 * TT, C], F32, tag="o")
                    st["o"] = o_sb
                o_sb = st["o"]
                for tt in range(TT):
                    f_ps = ps.tile([128, C], F32, tag="ps")
                    for ft in range(FT):
                        nc.tensor.matmul(
                            f_ps,
                            ff[:, ft, e, tt * 128:(tt + 1) * 128],
                            w2_sb[:, ft, :],
                            start=(ft == 0), stop=False)
                    nc.tensor.matmul(
                        f_ps, onesrow, brow_sb[:, B2, :],
                        start=False, stop=True)
                    i = e * TT + tt
                    nc.vector.tensor_add(out=o_sb[:, i, :],
                                         in0=x1[:, i, :], in1=f_ps)
                if e == 1:
                    nc.sync.dma_start(
                        out=outd[2 * p:2 * p + 2].rearrange(
                            "e (tt p) c -> p e tt c", p=128),
                        in_=o_sb)

            def emit_all():
                # two-pair software pipeline: pair p attention interleaves
                # with pair p-1 FFN so the PE always has cross-stream work
                states = {0: {}}
                s_ln1(0, states[0])
                for p in range(NP):
                    st = states[p]
                    prev = states.get(p - 1)
                    if prev is not None:
                        s_ln2(p - 1, prev)
                    s_tr1(p, st)
                    s_qkv(p, st)
                    if p + 1 < NP:
                        states[p + 1] = {}
                        s_ln1(p + 1, states[p + 1])
                    if prev is not None:
                        s_tr2(p - 1, prev)
                    s_attn(p, st, 0)
                    if prev is not None:
                        s_ffn1(p - 1, prev)
                    s_attn(p, st, 1)
                    s_pv(p, st, 0)
                    if prev is not None:
                        s_ffn2(p - 1, prev, 0)
                    s_pv(p, st, 1)
                    if prev is not None:
                        s_ffn2(p - 1, prev, 1)
                        del states[p - 1]
                    s_proj(p, st, 0)
                    s_proj(p, st, 1)
                last = states[NP - 1]
                s_ln2(NP - 1, last)
                s_tr2(NP - 1, last)
                s_ffn1(NP - 1, last)
                s_ffn2(NP - 1, last, 0)
                s_ffn2(NP - 1, last, 1)

            if loop_reps > 1:
                with tc.For_i(0, loop_reps, 1):
                    for _ in range(reps):
                        emit_all()
            else:
                for _ in range(reps):
                    emit_all()

    nc.compile()
    return nc


def _prep_maps(x, Wqkv, Wproj, bproj, W1, b1, W2, b2, g1, be1, g2, be2,
               nb=BL):
    import ml_dtypes
    f32 = np.float32
    f64 = np.float64
    bf16 = ml_dtypes.bfloat16
    Wqkv, Wproj = np.asarray(Wqkv, f64), np.asarray(Wproj, f64)
    W1, W2 = np.asarray(W1, f64), np.asarray(W2, f64)
    g1, be1 = np.asarray(g1, f64), np.asarray(be1, f64)
    g2, be2 = np.asarray(g2, f64), np.asarray(be2, f64)
    bproj, b1, b2 = (np.asarray(bproj, f64), np.asarray(b1, f64),
                     np.asarray(b2, f64))
    # fold LN gains into the consuming weights, LN betas into biases:
    #   h = z*g + be  =>  h @ W.T = z @ (W*g).T + (W @ be)
    Wqkv_g = Wqkv * g1[None, :]
    b_qkv = Wqkv @ be1                       # [3C]; q,k parts applied at copy
    bproj_eff = bproj + Wproj @ b_qkv[2 * C:]  # v bias folded via softmax sum=1
    W1_g = W1 * g2[None, :]
    b1_eff = b1 + W1 @ be2
    brows = np.stack([np.asarray(bproj_eff, f64),
                      np.asarray(b2, f64)]).astype(bf16)[None]  # [1,2,C]
    shared = {
        "wqkv": np.ascontiguousarray(Wqkv_g.astype(bf16).T).reshape(
            CT, 128, 3 * C),
        "wproj": np.ascontiguousarray(Wproj.astype(bf16).T).reshape(
            CT, 128, C),
        "w1": np.ascontiguousarray(W1_g.astype(bf16).T).reshape(
            CT, 128, 4 * C),
        "w2": np.ascontiguousarray(W2.astype(bf16).T).reshape(FT, 128, C),
        "b1": np.ascontiguousarray(b1_eff.astype(f32).reshape(FT, 128)),
        "qkb": np.ascontiguousarray(b_qkv[:2 * C].astype(f32).reshape(
            2 * CT, 128)),
        "brows": brows,
    }
    x = np.asarray(x, f32)
    return [dict(shared, x=np.ascontiguousarray(x[i * nb:(i + 1) * nb]))
            for i in range(NCORES)]


def run(inputs, reps=1, trace=False, nb=BL):
    from concourse import bass_utils
    key = ("nc", reps, nb)
    if key not in _cache:
        _cache[key] = build(reps, nb)
    nc = _cache[key]
    in_maps = _prep_maps(**inputs, nb=nb)
    res = bass_utils.run_bass_kernel_spmd(
        nc, in_maps, core_ids=list(range(NCORES)), trace=trace)
    out = np.concatenate([res.results[i]["out"] for i in range(NCORES)], axis=0)
    return out, res


def kernel(**inputs):
    out, _ = run(inputs)
    return out


# ---------- cached jitted runner for benchmarking (execute-only calls) ----------
def get_runner(reps=1, nb=BL, loop_reps=1, skip=()):
    """Returns (call, put) where put(in_maps) -> device args and call(args)
    executes the prebuilt NEFF on 8 cores, returning jax output arrays.
    Mirrors bass2jax.run_bass_via_pjrt but with a persistent jit cache."""
    import jax
    import numpy as _np
    from jax.experimental.shard_map import shard_map
    from jax.sharding import Mesh, PartitionSpec, NamedSharding
    from concourse import bass2jax as B2J
    import concourse.mybir as mybir

    key = ("runner", reps, nb, loop_reps, tuple(skip))
    if key in _cache:
        return _cache[key]
    nckey = ("nc", reps, nb, loop_reps, tuple(skip))
    if nckey not in _cache:
        _cache[nckey] = build(reps, nb, loop_reps=loop_reps, skip=skip)
    nc = _cache[nckey]

    B2J.install_neuronx_cc_hook()
    part_name = (nc.partition_id_tensor.name if nc.partition_id_tensor
                 else None)
    in_names, out_names, out_avals, zero_outs = [], [], [], []
    for alloc in nc.m.functions[0].allocations:
        if not isinstance(alloc, mybir.MemoryLocationSet):
            continue
        name = alloc.memorylocations[0].name
        if alloc.kind == "ExternalInput":
            if name != part_name:
                in_names.append(name)
        elif alloc.kind == "ExternalOutput":
            out_names.append(name)
            shape = tuple(alloc.tensor_shape)
            dtype = mybir.dt.np(alloc.dtype)
            out_avals.append(jax.core.ShapedArray(shape, dtype))
            zero_outs.append(_np.zeros(shape, dtype))
    n_params = len(in_names)
    all_names = in_names + out_names
    if part_name is not None:
        all_names = all_names + [part_name]

    def _body(*args):
        operands = list(args)
        if part_name is not None:
            operands.append(B2J.partition_id_tensor())
        outs = B2J._bass_exec_p.bind(
            *operands,
            out_avals=tuple(out_avals),
            in_names=tuple(all_names),
            out_names=tuple(out_names),
            lowering_input_output_aliases=(),
            sim_require_finite=True,
            sim_require_nnan=True,
            nc=nc,
        )
        return tuple(outs)

    devices = jax.devices()[:NCORES]
    mesh = Mesh(_np.asarray(devices), ("core",))
    spec = PartitionSpec("core")
    n_outs = len(out_names)
    sharded = jax.jit(
        shard_map(_body, mesh=mesh, in_specs=(spec,) * (n_params + n_outs),
                  out_specs=(spec,) * n_outs, check_rep=False),
        keep_unused=True)
    sharding = NamedSharding(mesh, spec)

    def put(in_maps):
        args = []
        for i, name in enumerate(in_names):
            cat = _np.concatenate([_np.asarray(m[name]) for m in in_maps], 0)
            args.append(jax.device_put(cat, sharding))
        for z in zero_outs:
            cat = _np.zeros((NCORES * z.shape[0], *z.shape[1:]), z.dtype)
            args.append(jax.device_put(cat, sharding))
        return args

    def call(args):
        outs = sharded(*args)
        jax.block_until_ready(outs)
        return outs

    _cache[key] = (call, put)
    return call, put
